# revision 28
# baseline (speedup 1.0000x reference)
"""Trainium2 Bass kernel for nn_EncodingModel (GNN message-passing scorer).

logits[i, j] = p_hat[i].w_p + ns_j + p_hat[i].cross_j + b + filt[j]
  cross_j = sum_s n_hat[nbr_s(j)] * w_c_s      (slot 0 = self)
  ns_j    = sum_s n_hat[nbr_s(j)] . w_n_s      (host-precomputed)

Sharding: nodes (axis 0) across 8 cores; phrases and weights replicated.

Host prep: node/phrase embeddings are L2-normalized on host; the node table
is stored bf16 as PAIRS (row k = nodes 2k,2k+1) so one descriptor per
gathered item stays in dma_gather's int16 index range; a DVE uint32-view
copy + predicated copy select the wanted half. The per-node scalar term
ns+filt+b is folded into one f32 bias vector; phrases arrive as p_hatT.

Masked-slot skipping: each core's nodes are sorted by neighbor-mask pattern
(output unpermuted on host), making most 128-node tiles mask-pattern-pure.
A (tile, slot) column whose mask bit is clear for the whole tile is dropped
from the gather AND from all downstream compute; the kernel program is
built per-input from the resulting column spec (~halves gather descriptors,
which are the binding resource at ~150ns/descriptor/engine).

Per-core pipeline over chunks of 4 node-tiles (processed light->heavy->
light so the pipeline fills fast and drains short; gathers split in two
sub-calls on different SWDGE queues to overlap emission with transfer):
  - one dma_gather of bf16 pair rows for the chunk's kept columns
  - uint32-view copy + predicated copy select pair halves into gbuf;
    self rows stream in via direct DMA
  - DVE mults apply w_c per slot-range -> z (bf16)
  - PE accumulating matmuls z.T @ I build crossT[d, node] in PSUM (the
    slot-sum happens in PSUM); ACT evacuates with bias=w_p[d]
  - main matmuls crossT.T @ p_hatT give out[node, phrase] in PSUM; ACT
    (DVE for the tail chunks) evacuates with bias=nsf[node] into bf16
  - HWDGE store of [128 x 1024] bf16 rows; host transposes to [P, N] f32
"""

import numpy as np
import ml_dtypes

import concourse.bass as bass
import concourse.bacc as bacc
import concourse.mybir as mybir
from concourse.bass_utils import run_bass_kernel_spmd
from concourse.tile import TileContext

F32 = mybir.dt.float32
BF16 = mybir.dt.bfloat16
I16 = mybir.dt.int16
U8 = mybir.dt.uint8
U32 = mybir.dt.uint32
F8 = mybir.dt.float8e4
OP = mybir.AluOpType
AF = mybir.ActivationFunctionType

N_NODES = 50000
N_PHRASES = 1024
D = 256
R = 4
SLOTS = 1 + R
NEG_INF = -999999.0
N_CORES = 8
NS = N_NODES // N_CORES            # 6250 nodes per core
TP_ROWS = 25088                    # pair rows; nodes 50000..50175 are zero
CHUNK_TILES = 2

# pattern order for the node sort: chain patterns so adjacent ones share
# bits (boundary tiles keep the union of their patterns' slots)
PAT_ORDER = [0, 1, 3, 2, 6, 7, 5, 4, 12, 13, 15, 14, 10, 11, 9, 8]
PAT_RANK = np.zeros(16, dtype=np.int64)
for _i, _p in enumerate(PAT_ORDER):
    PAT_RANK[_p] = _i


def build_kernel(n_tiles, chunk_specs):
    """Build the SPMD Bass program.

    chunk_specs: tuple of (ct0, nt, cols) where cols is a tuple of
    (tile_local, slot) kept neighbor columns, sorted slot-major.
    """
    nc = bacc.Bacc(None, target_bir_lowering=False, num_swdge_queues=4)

    ns_pad = n_tiles * 128
    total_kept = sum(len(cols) for _, _, cols in chunk_specs)
    n_idx_cols = max(total_kept * 8, 16)
    tpair = nc.declare_dram_parameter("tpair", [TP_ROWS, 2 * D], BF16,
                                      isOutput=False)
    tself = nc.declare_dram_parameter("tself", [ns_pad, D], BF16, isOutput=False)
    pT_d = nc.declare_dram_parameter("pT", [128, 2 * N_PHRASES], BF16, isOutput=False)
    wcb_d = nc.declare_dram_parameter("wcb", [128, SLOTS * D], BF16, isOutput=False)
    wpb_d = nc.declare_dram_parameter("wpb", [128, 2], F32, isOutput=False)
    identb_d = nc.declare_dram_parameter("identb", [128, 128], BF16, isOutput=False)
    ipr_d = nc.declare_dram_parameter("ipr", [128, n_idx_cols], I16, isOutput=False)
    oddm_d = nc.declare_dram_parameter("oddm", [128, max(total_kept, 1)], U8,
                                       isOutput=False)
    nsf_d = nc.declare_dram_parameter("nsf", [128, n_tiles], F32, isOutput=False)
    out_d = nc.declare_dram_parameter("out", [ns_pad, N_PHRASES], BF16, isOutput=True)

    with TileContext(nc) as tc:
        with tc.tile_pool(name="const", bufs=1) as cpool:
            # ---- constants (index tables first so gathers start early) ----
            ipr_t = cpool.tile([128, n_idx_cols], I16, tag="ipr")
            nc.sync.dma_start(out=ipr_t[:], in_=ipr_d[:])
            oddm_t = cpool.tile([128, max(total_kept, 1)], U8, tag="oddm")
            nc.sync.dma_start(out=oddm_t[:], in_=oddm_d[:])
            pT_t = cpool.tile([128, 2 * N_PHRASES], BF16, tag="pT")
            nc.sync.dma_start(out=pT_t[:], in_=pT_d[:])
            wcb = cpool.tile([128, SLOTS * D], BF16, tag="wcb")
            nc.sync.dma_start(out=wcb[:], in_=wcb_d[:])
            wpb = cpool.tile([128, 2], F32, tag="wpb")
            nc.sync.dma_start(out=wpb[:], in_=wpb_d[:])
            identb = cpool.tile([128, 128], BF16, tag="identb")
            nc.sync.dma_start(out=identb[:], in_=identb_d[:])
            nsf_t = cpool.tile([128, n_tiles], F32, tag="nsf")
            nc.sync.dma_start(out=nsf_t[:], in_=nsf_d[:])

            wux = cpool.tile([128, 16], I16, tag="wux")
            nc.gpsimd.memset(wux[:], 0)
            wug = cpool.tile([128, 2, 2 * D], BF16, tag="wug")
            nc.gpsimd.dma_gather(wug[:], tpair[:], wux[:], 256, 256, 2 * D,
                                 single_packet=False, queue_num=3)

            # ---- main loop over node chunks ----
            with (
                tc.tile_pool(name="gio", bufs=5) as giop,
                tc.tile_pool(name="gather", bufs=4) as gpool,
                tc.tile_pool(name="zb", bufs=3) as zpool,
                tc.tile_pool(name="xT", bufs=3) as xpool,
                tc.tile_pool(name="lout", bufs=4) as lpool,
                tc.tile_pool(name="pm_ct", bufs=2, space="PSUM") as pm_ct,
                tc.tile_pool(name="pm_l", bufs=2, space="PSUM") as pm_l,
            ):
              koff = 0
              for ci, (ct0, nt, cols) in enumerate(chunk_specs):
                  nw = nt * 128
                  nkept = len(cols)
                  ncols = nkept + nt          # kept nbr columns, then self
                  nidx = nkept * 128
                  gbuf = gpool.tile([128, ncols, D], BF16, tag="gbuf",
                                    name=f"gbuf{ct0}")
                  if nkept:
                      pb = giop.tile([128, nkept, 2 * D], BF16, tag="pb",
                                     name=f"pb{ct0}")
                      k2 = (nkept + 1) // 2 if nkept >= 6 else nkept
                      parts = [(0, k2)] + ([(k2, nkept)] if k2 < nkept else [])
                      for gi, (ka, kb) in enumerate(parts):
                          nn = (kb - ka) * 128
                          nc.gpsimd.dma_gather(
                              pb[:, ka:kb, :], tpair[:],
                              ipr_t[:, (koff + ka) * 8:(koff + kb) * 8],
                              nn, nn, 2 * D, single_packet=False,
                              queue_num=(2 * ci + gi) % 4)
                      nc.vector.tensor_copy(gbuf[:, 0:nkept, :].bitcast(U32),
                                            pb[:, :, 0:D].bitcast(U32))
                      nc.vector.copy_predicated(
                          gbuf[:, 0:nkept, :].bitcast(U32),
                          oddm_t[:, koff:koff + nkept]
                              .rearrange("p (c o) -> p c o", o=1)
                              .to_broadcast([128, nkept, D // 2]),
                          pb[:, :, D:2 * D].bitcast(U32))
                  nc.sync.dma_start(
                      out=gbuf[:, nkept:ncols, :],
                      in_=tself[:].rearrange("(t p) d -> p t d", p=128)
                          [:, ct0:ct0 + nt, :])

                  # z = x * w_c (bf16, x128 scaled); per-slot column ranges
                  z = zpool.tile([128, ncols, D], BF16, tag="z")
                  c0 = 0
                  while c0 < nkept:
                      s = cols[c0][1]
                      c1 = c0
                      while c1 < nkept and cols[c1][1] == s:
                          c1 += 1
                      nc.vector.tensor_tensor(
                          out=z[:, c0:c1, :],
                          in0=gbuf[:, c0:c1, :],
                          in1=wcb[:, s * D:(s + 1) * D]
                              .rearrange("p (o d) -> p o d", o=1)
                              .to_broadcast([128, c1 - c0, D]),
                          op=OP.mult)
                      c0 = c1
                  nc.vector.tensor_tensor(
                      out=z[:, nkept:ncols, :],
                      in0=gbuf[:, nkept:ncols, :],
                      in1=wcb[:, 0:D]
                          .rearrange("p (o d) -> p o d", o=1)
                          .to_broadcast([128, nt, D]),
                      op=OP.mult)

                  # crossT accumulation in PSUM: crossT[d, node] = sum_s z_s
                  # tile-major emission: each tile's accumulation group is
                  # contiguous (start=True clears has_written bits for the
                  # WHOLE bank, so tile groups must not interleave); the
                  # self column comes last per tile and carries stop
                  psum_ct = [pm_ct.tile([128, nw], F32, space="PSUM", tag="pm_ct",
                                        name=f"psum_ct{ct0}_{h}") for h in range(2)]
                  for tl in range(nt):
                      tcols = [ci2 for ci2 in range(nkept)
                               if cols[ci2][0] == tl] + [nkept + tl]
                      for k, ci2 in enumerate(tcols):
                          for h in range(2):
                              nc.tensor.matmul(
                                  psum_ct[h][:, tl * 128:(tl + 1) * 128],
                                  lhsT=z[:, ci2, h * 128:(h + 1) * 128],
                                  rhs=identb[:],
                                  start=(k == 0), stop=(k == len(tcols) - 1))

                  # evac crossT: scale 1/128 (fp8 scales) + bias w_p[d]
                  crossT = [xpool.tile([128, nw], BF16, tag=f"crossT{h}",
                                       name=f"crossT{ct0}_{h}") for h in range(2)]
                  for h in range(2):
                      nc.scalar.add(crossT[h][:], psum_ct[h][:],
                                    add=wpb[:, h:h + 1])

                  # main matmuls: out[node, phrase] = crossT'.T @ p_hatT
                  for tl in range(nt):
                      jsl = slice(tl * 128, (tl + 1) * 128)
                      lsb = lpool.tile([128, N_PHRASES], BF16, tag="lsb")
                      psl = pm_l.tile([128, N_PHRASES], F32, space="PSUM",
                                      tag="pm_l")
                      for ih in range(2):
                          for h in range(2):
                              nc.tensor.matmul(
                                  psl[:, ih * 512:(ih + 1) * 512],
                                  lhsT=crossT[h][:, jsl],
                                  rhs=pT_t[:, h * N_PHRASES + ih * 512:
                                           h * N_PHRASES + (ih + 1) * 512],
                                  start=(h == 0), stop=(h == 1))
                      if ci >= len(chunk_specs) - 5:
                          nc.vector.tensor_tensor(
                              out=lsb[:].rearrange("p (o e) -> p o e", o=1),
                              in0=psl[:].rearrange("p (o e) -> p o e", o=1),
                              in1=nsf_t[:, ct0 + tl:ct0 + tl + 1]
                                  .rearrange("p (o e) -> p o e", o=1)
                                  .to_broadcast([128, 1, N_PHRASES]),
                              op=OP.add)
                      else:
                          nc.scalar.add(lsb[:], psl[:],
                                        add=nsf_t[:, ct0 + tl:ct0 + tl + 1])
                      nc.sync.dma_start(
                          out=out_d[(ct0 + tl) * 128:(ct0 + tl + 1) * 128, :],
                          in_=lsb[:])
                  koff += nkept

    nc.finalize()
    return nc


def _plan_core(idx_eff, pat, base, cover, n_tiles):
    """Sort a core's nodes by mask pattern; derive tiles and kept columns.

    Returns (gids, chunk_specs, col_ipr, col_oddm) where gids[r] is the
    global node id at permuted row r (-1 for pad rows) and col_ipr/col_oddm
    are dicts (chunk_i, tile_local, slot) -> [128] arrays.
    """
    ns_pad = n_tiles * 128
    ids = np.arange(base, base + cover)
    perm = np.argsort(PAT_RANK[pat[ids]], kind="stable")
    gids = np.full(ns_pad, -1, dtype=np.int64)
    gids[:cover] = ids[perm]

    pat_pad = np.zeros(ns_pad, dtype=np.int64)
    pat_pad[:cover] = pat[gids[:cover]]
    tile_mask = np.bitwise_or.reduce(pat_pad.reshape(n_tiles, 128), axis=1)

    nbr = np.full((ns_pad, R), N_NODES, dtype=np.int64)
    nbr[:cover] = idx_eff[gids[:cover]]

    chunk_specs = []
    col_ipr = {}
    col_oddm = {}
    t0 = 0
    chunk_i = 0
    while t0 < n_tiles:
        nt = min(CHUNK_TILES, n_tiles - t0)
        cols = []
        for s in range(1, SLOTS):
            for tl in range(nt):
                if tile_mask[t0 + tl] & (1 << (s - 1)):
                    cols.append((tl, s))
        for tl, s in cols:
            rows = nbr[(t0 + tl) * 128:(t0 + tl + 1) * 128, s - 1]
            col_ipr[(t0, tl, s)] = rows >> 1
            col_oddm[(t0, tl, s)] = (rows & 1).astype(np.uint8)
        chunk_specs.append((t0, nt, tuple(cols)))
        t0 += nt
        chunk_i += 1
    return gids, tuple(chunk_specs), col_ipr, col_oddm


def _host_inputs(node_embeddings, phrase_embeddings, score_w, score_b,
                 neighbors, neighbor_mask, node_filter_mask, n_tiles):
    """Build per-core input maps + merged chunk specs + row->node maps."""
    ns_pad = n_tiles * 128
    cover = min(NS, ns_pad)

    n32 = node_embeddings.astype(np.float32)
    nrm = np.sqrt((n32 * n32).sum(axis=1, keepdims=True))
    nhat = n32 / np.maximum(nrm, 1e-8)
    tb = nhat.astype(ml_dtypes.bfloat16)
    tpair = np.zeros((TP_ROWS, 2 * D), dtype=ml_dtypes.bfloat16)
    tpair.reshape(-1, D)[:N_NODES] = tb

    p32 = phrase_embeddings.astype(np.float32)
    prm = np.sqrt((p32 * p32).sum(axis=1, keepdims=True))
    phat = p32 / np.maximum(prm, 1e-8)
    pT = (phat.astype(ml_dtypes.bfloat16).T
          .reshape(2, 128, N_PHRASES).transpose(1, 0, 2)
          .reshape(128, 2 * N_PHRASES)).copy()

    w_p = score_w[:D].astype(np.float32)
    rest = score_w[D:].reshape(SLOTS, 2, D).astype(np.float32)
    w_n, w_c = rest[:, 0, :], rest[:, 1, :]
    wcb = np.broadcast_to(
        w_c.reshape(-1).astype(ml_dtypes.bfloat16), (128, SLOTS * D)).copy()
    wpb = np.ascontiguousarray(w_p.reshape(2, 128).T)  # [p, h]

    identb = np.eye(128, dtype=ml_dtypes.bfloat16)

    idx_eff = np.where(neighbor_mask > 0, neighbors, N_NODES).astype(np.int64)
    pat = (np.minimum(neighbor_mask, 1).astype(np.int64)
           << np.arange(R)).sum(axis=1)

    dn = nhat @ w_n.T                                     # [N, SLOTS]
    nsv = dn[:, 0].copy()
    for s in range(R):
        nsv += np.where(neighbor_mask[:, s] > 0,
                        dn[neighbors[:, s], s + 1], 0.0).astype(np.float32)
    filt = np.where(node_filter_mask > 0, 0.0, NEG_INF).astype(np.float32)
    nsf = (nsv + filt + np.float32(score_b)).astype(np.float32)

    plans = []
    for c in range(N_CORES):
        plans.append(_plan_core(idx_eff, pat, c * NS, cover, n_tiles))

    # SPMD: all cores share one program; per chunk use the union of kept
    # columns across cores (a column absent for a core gathers zero pairs)
    merged = []
    for chunk_i in range(len(plans[0][1])):
        ct0, nt, _ = plans[0][1][chunk_i]
        colset = set()
        for c in range(N_CORES):
            colset.update(plans[c][1][chunk_i][2])
        cols = tuple(sorted(colset, key=lambda ts: (ts[1], ts[0])))
        merged.append((ct0, nt, cols))
    # pyramid order: light chunks first (fast pipeline start), heavy in the
    # middle, light at the end (short drain)
    asc = sorted(merged, key=lambda e: len(e[2]))
    merged = tuple(asc[0::2] + asc[1::2][::-1])

    def wrap_idx(flat):
        blk = flat.reshape(-1, 16).T.astype(np.int16)      # [16, n/16]
        return np.tile(blk, (8, 1))

    zero_ipr = np.full(128, N_NODES >> 1, dtype=np.int64)
    zero_oddm = np.zeros(128, dtype=np.uint8)

    in_maps = []
    gid_maps = []
    for c in range(N_CORES):
        gids, _, col_ipr, col_oddm = plans[c]
        gid_maps.append(gids)

        ipr_parts = []
        oddm_parts = []
        for ct0, nt, cols in merged:
            for tl, s in cols:
                ipr_parts.append(col_ipr.get((ct0, tl, s), zero_ipr))
                oddm_parts.append(col_oddm.get((ct0, tl, s), zero_oddm))
        if ipr_parts:
            ipr_flat = np.concatenate(ipr_parts)
            oddm_cols = np.stack(oddm_parts, axis=1)
        else:
            ipr_flat = np.zeros(128, dtype=np.int64)
            oddm_cols = np.zeros((128, 1), dtype=np.uint8)

        tsf = np.zeros((ns_pad, D), dtype=tb.dtype)
        tsf[gids >= 0] = tb[gids[gids >= 0]]

        fb = np.zeros(ns_pad, dtype=np.float32)
        fb[gids >= 0] = nsf[gids[gids >= 0]]
        fb_tile = fb.reshape(n_tiles, 128).transpose(1, 0).copy()

        in_maps.append({
            "tpair": tpair,
            "tself": tsf,
            "pT": pT,
            "wcb": wcb,
            "wpb": wpb,
            "identb": identb,
            "ipr": wrap_idx(ipr_flat),
            "oddm": np.ascontiguousarray(oddm_cols),
            "nsf": np.ascontiguousarray(fb_tile),
        })
    return in_maps, merged, gid_maps


_CACHE = {}


def run_sharded(node_embeddings, phrase_embeddings, score_w, score_b,
                neighbors, neighbor_mask, node_filter_mask,
                n_tiles=None, trace=False):
    if n_tiles is None:
        n_tiles = (NS + 127) // 128  # 49
    in_maps, merged, gid_maps = _host_inputs(
        node_embeddings, phrase_embeddings, score_w, score_b,
        neighbors, neighbor_mask, node_filter_mask, n_tiles)
    key = (n_tiles, merged)
    if key not in _CACHE:
        _CACHE.clear()
        _CACHE[key] = build_kernel(n_tiles, merged)
    nc = _CACHE[key]
    res = run_bass_kernel_spmd(nc, in_maps, list(range(N_CORES)), trace=trace)

    full_t = np.zeros((N_NODES, N_PHRASES), dtype=np.float32)
    for c in range(N_CORES):
        gids = gid_maps[c]
        valid = gids >= 0
        full_t[gids[valid]] = res.results[c]["out"][valid]
    out = np.ascontiguousarray(full_t.T)
    return out, res


def kernel(node_embeddings, phrase_embeddings, score_w, score_b,
           neighbors, neighbor_mask, node_filter_mask):
    out, _ = run_sharded(
        np.asarray(node_embeddings, dtype=np.float32),
        np.asarray(phrase_embeddings, dtype=np.float32),
        np.asarray(score_w, dtype=np.float32),
        np.asarray(score_b, dtype=np.float32),
        np.asarray(neighbors),
        np.asarray(neighbor_mask),
        np.asarray(node_filter_mask))
    return out


# revision 30
# speedup vs baseline: 1.0695x; 1.0695x over previous
"""Trainium2 Bass kernel for nn_EncodingModel (GNN message-passing scorer).

logits[i, j] = p_hat[i].w_p + ns_j + p_hat[i].cross_j + b + filt[j]
  cross_j = sum_s n_hat[nbr_s(j)] * w_c_s      (slot 0 = self)
  ns_j    = sum_s n_hat[nbr_s(j)] . w_n_s      (host-precomputed)

Sharding: nodes (axis 0) across 8 cores; phrases and weights replicated.

Host prep: node/phrase embeddings are L2-normalized on host; the node table
is stored bf16 as PAIRS (row k = nodes 2k,2k+1) so one descriptor per
gathered item stays in dma_gather's int16 index range; a DVE uint32-view
copy + predicated copy select the wanted half. The per-node scalar term
ns+filt+b is folded into one f32 bias vector; phrases arrive as p_hatT.

Masked-slot skipping: each core's nodes are sorted by neighbor-mask pattern
(output unpermuted on host), making most 128-node tiles mask-pattern-pure.
A (tile, slot) column whose mask bit is clear for the whole tile is dropped
from the gather AND from all downstream compute; the kernel program is
built per-input from the resulting column spec (~halves gather descriptors,
which are the binding resource at ~150ns/descriptor/engine).

Per-core pipeline over chunks of 4 node-tiles (processed light->heavy->
light so the pipeline fills fast and drains short; gathers split in two
sub-calls on different SWDGE queues to overlap emission with transfer):
  - one dma_gather of bf16 pair rows for the chunk's kept columns
  - uint32-view copy + predicated copy select pair halves into gbuf;
    self rows stream in via direct DMA
  - DVE mults apply w_c per slot-range -> z (bf16)
  - PE accumulating matmuls z.T @ I build crossT[d, node] in PSUM (the
    slot-sum happens in PSUM); ACT evacuates with bias=w_p[d]
  - main matmuls crossT.T @ p_hatT give out[node, phrase] in PSUM; ACT
    (DVE for the tail chunks) evacuates with bias=nsf[node] into bf16
  - HWDGE store of [128 x 1024] bf16 rows; host transposes to [P, N] f32
"""

import numpy as np
import ml_dtypes

import concourse.bass as bass
import concourse.bacc as bacc
import concourse.mybir as mybir
from concourse.bass_utils import run_bass_kernel_spmd
from concourse.tile import TileContext

F32 = mybir.dt.float32
BF16 = mybir.dt.bfloat16
I16 = mybir.dt.int16
U8 = mybir.dt.uint8
U32 = mybir.dt.uint32
F8 = mybir.dt.float8e4
OP = mybir.AluOpType
AF = mybir.ActivationFunctionType

N_NODES = 50000
N_PHRASES = 1024
D = 256
R = 4
SLOTS = 1 + R
NEG_INF = -999999.0
N_CORES = 8
NS = N_NODES // N_CORES            # 6250 nodes per core
TP_ROWS = 25088                    # pair rows; nodes 50000..50175 are zero
CHUNK_TILES = 4

# pattern order for the node sort: chain patterns so adjacent ones share
# bits (boundary tiles keep the union of their patterns' slots)
PAT_ORDER = [0, 1, 3, 2, 6, 7, 5, 4, 12, 13, 15, 14, 10, 11, 9, 8]
PAT_RANK = np.zeros(16, dtype=np.int64)
for _i, _p in enumerate(PAT_ORDER):
    PAT_RANK[_p] = _i


def build_kernel(n_tiles, chunk_specs):
    """Build the SPMD Bass program.

    chunk_specs: tuple of (ct0, nt, cols) where cols is a tuple of
    (tile_local, slot) kept neighbor columns, sorted slot-major.
    """
    nc = bacc.Bacc(None, target_bir_lowering=False, num_swdge_queues=4)

    ns_pad = n_tiles * 128
    total_kept = sum(len(cols) for _, _, cols in chunk_specs)
    n_idx_cols = max(total_kept * 8, 16)
    tpair = nc.declare_dram_parameter("tpair", [TP_ROWS, 2 * D], BF16,
                                      isOutput=False)
    tself = nc.declare_dram_parameter("tself", [ns_pad, D], BF16, isOutput=False)
    pT_d = nc.declare_dram_parameter("pT", [128, 2 * N_PHRASES], BF16, isOutput=False)
    wcb_d = nc.declare_dram_parameter("wcb", [128, SLOTS * D], BF16, isOutput=False)
    wpb_d = nc.declare_dram_parameter("wpb", [128, 2], F32, isOutput=False)
    identb_d = nc.declare_dram_parameter("identb", [128, 128], BF16, isOutput=False)
    ipr_d = nc.declare_dram_parameter("ipr", [128, n_idx_cols], I16, isOutput=False)
    oddm_d = nc.declare_dram_parameter("oddm", [128, max(total_kept, 1)], U8,
                                       isOutput=False)
    nsf_d = nc.declare_dram_parameter("nsf", [128, n_tiles], F32, isOutput=False)
    out_d = nc.declare_dram_parameter("out", [ns_pad, N_PHRASES], BF16, isOutput=True)

    with TileContext(nc) as tc:
        with tc.tile_pool(name="const", bufs=1) as cpool:
            # ---- constants (index tables first so gathers start early) ----
            ipr_t = cpool.tile([128, n_idx_cols], I16, tag="ipr")
            nc.sync.dma_start(out=ipr_t[:], in_=ipr_d[:])
            oddm_t = cpool.tile([128, max(total_kept, 1)], U8, tag="oddm")
            nc.sync.dma_start(out=oddm_t[:], in_=oddm_d[:])
            pT_t = cpool.tile([128, 2 * N_PHRASES], BF16, tag="pT")
            nc.sync.dma_start(out=pT_t[:], in_=pT_d[:])
            wcb = cpool.tile([128, SLOTS * D], BF16, tag="wcb")
            nc.sync.dma_start(out=wcb[:], in_=wcb_d[:])
            wpb = cpool.tile([128, 2], F32, tag="wpb")
            nc.sync.dma_start(out=wpb[:], in_=wpb_d[:])
            identb = cpool.tile([128, 128], BF16, tag="identb")
            nc.sync.dma_start(out=identb[:], in_=identb_d[:])
            nsf_t = cpool.tile([128, n_tiles], F32, tag="nsf")
            nc.sync.dma_start(out=nsf_t[:], in_=nsf_d[:])

            wux = cpool.tile([128, 16], I16, tag="wux")
            nc.gpsimd.memset(wux[:], 0)
            wug = cpool.tile([128, 2, 2 * D], BF16, tag="wug")
            nc.gpsimd.dma_gather(wug[:], tpair[:], wux[:], 256, 256, 2 * D,
                                 single_packet=False, queue_num=3)

            # ---- main loop over node chunks ----
            with (
                tc.tile_pool(name="gio", bufs=5) as giop,
                tc.tile_pool(name="gather", bufs=4) as gpool,
                tc.tile_pool(name="zb", bufs=3) as zpool,
                tc.tile_pool(name="xT", bufs=3) as xpool,
                tc.tile_pool(name="lout", bufs=4) as lpool,
                tc.tile_pool(name="pm_ct", bufs=2, space="PSUM") as pm_ct,
                tc.tile_pool(name="pm_l", bufs=2, space="PSUM") as pm_l,
            ):
              koff = 0
              for ci, (ct0, nt, cols) in enumerate(chunk_specs):
                  nw = nt * 128
                  nkept = len(cols)
                  ncols = nkept + nt          # kept nbr columns, then self
                  nidx = nkept * 128
                  gbuf = gpool.tile([128, ncols, D], BF16, tag="gbuf",
                                    name=f"gbuf{ct0}")
                  if nkept:
                      pb = giop.tile([128, nkept, 2 * D], BF16, tag="pb",
                                     name=f"pb{ct0}")
                      nparts = max(1, (nkept + 3) // 4)
                      bounds = [nkept * i // nparts for i in range(nparts + 1)]
                      parts = list(zip(bounds[:-1], bounds[1:]))
                      for gi, (ka, kb) in enumerate(parts):
                          nn = (kb - ka) * 128
                          nc.gpsimd.dma_gather(
                              pb[:, ka:kb, :], tpair[:],
                              ipr_t[:, (koff + ka) * 8:(koff + kb) * 8],
                              nn, nn, 2 * D, single_packet=False,
                              queue_num=(2 * ci + gi) % 4)
                      # per-part merge: each select starts as soon as its
                      # own sub-gather lands instead of waiting for all
                      for ka, kb in parts:
                          nc.vector.tensor_copy(
                              gbuf[:, ka:kb, :].bitcast(U32),
                              pb[:, ka:kb, 0:D].bitcast(U32))
                          nc.vector.copy_predicated(
                              gbuf[:, ka:kb, :].bitcast(U32),
                              oddm_t[:, koff + ka:koff + kb]
                                  .rearrange("p (c o) -> p c o", o=1)
                                  .to_broadcast([128, kb - ka, D // 2]),
                              pb[:, ka:kb, D:2 * D].bitcast(U32))
                  nc.sync.dma_start(
                      out=gbuf[:, nkept:ncols, :],
                      in_=tself[:].rearrange("(t p) d -> p t d", p=128)
                          [:, ct0:ct0 + nt, :])

                  # z = x * w_c (bf16, x128 scaled); per-slot column ranges
                  z = zpool.tile([128, ncols, D], BF16, tag="z")
                  c0 = 0
                  while c0 < nkept:
                      s = cols[c0][1]
                      c1 = c0
                      while c1 < nkept and cols[c1][1] == s:
                          c1 += 1
                      nc.vector.tensor_tensor(
                          out=z[:, c0:c1, :],
                          in0=gbuf[:, c0:c1, :],
                          in1=wcb[:, s * D:(s + 1) * D]
                              .rearrange("p (o d) -> p o d", o=1)
                              .to_broadcast([128, c1 - c0, D]),
                          op=OP.mult)
                      c0 = c1
                  nc.vector.tensor_tensor(
                      out=z[:, nkept:ncols, :],
                      in0=gbuf[:, nkept:ncols, :],
                      in1=wcb[:, 0:D]
                          .rearrange("p (o d) -> p o d", o=1)
                          .to_broadcast([128, nt, D]),
                      op=OP.mult)

                  # crossT accumulation in PSUM: crossT[d, node] = sum_s z_s
                  # tile-major emission: each tile's accumulation group is
                  # contiguous (start=True clears has_written bits for the
                  # WHOLE bank, so tile groups must not interleave); the
                  # self column comes last per tile and carries stop
                  psum_ct = [pm_ct.tile([128, nw], F32, space="PSUM", tag="pm_ct",
                                        name=f"psum_ct{ct0}_{h}") for h in range(2)]
                  for tl in range(nt):
                      tcols = [ci2 for ci2 in range(nkept)
                               if cols[ci2][0] == tl] + [nkept + tl]
                      for k, ci2 in enumerate(tcols):
                          for h in range(2):
                              nc.tensor.matmul(
                                  psum_ct[h][:, tl * 128:(tl + 1) * 128],
                                  lhsT=z[:, ci2, h * 128:(h + 1) * 128],
                                  rhs=identb[:],
                                  start=(k == 0), stop=(k == len(tcols) - 1))

                  # evac crossT: scale 1/128 (fp8 scales) + bias w_p[d]
                  crossT = [xpool.tile([128, nw], BF16, tag=f"crossT{h}",
                                       name=f"crossT{ct0}_{h}") for h in range(2)]
                  for h in range(2):
                      nc.scalar.add(crossT[h][:], psum_ct[h][:],
                                    add=wpb[:, h:h + 1])

                  # main matmuls: out[node, phrase] = crossT'.T @ p_hatT
                  for tl in range(nt):
                      jsl = slice(tl * 128, (tl + 1) * 128)
                      lsb = lpool.tile([128, N_PHRASES], BF16, tag="lsb")
                      psl = pm_l.tile([128, N_PHRASES], F32, space="PSUM",
                                      tag="pm_l")
                      for ih in range(2):
                          for h in range(2):
                              nc.tensor.matmul(
                                  psl[:, ih * 512:(ih + 1) * 512],
                                  lhsT=crossT[h][:, jsl],
                                  rhs=pT_t[:, h * N_PHRASES + ih * 512:
                                           h * N_PHRASES + (ih + 1) * 512],
                                  start=(h == 0), stop=(h == 1))
                      if ci >= len(chunk_specs) - 5:
                          nc.vector.tensor_tensor(
                              out=lsb[:].rearrange("p (o e) -> p o e", o=1),
                              in0=psl[:].rearrange("p (o e) -> p o e", o=1),
                              in1=nsf_t[:, ct0 + tl:ct0 + tl + 1]
                                  .rearrange("p (o e) -> p o e", o=1)
                                  .to_broadcast([128, 1, N_PHRASES]),
                              op=OP.add)
                      else:
                          nc.scalar.add(lsb[:], psl[:],
                                        add=nsf_t[:, ct0 + tl:ct0 + tl + 1])
                      nc.sync.dma_start(
                          out=out_d[(ct0 + tl) * 128:(ct0 + tl + 1) * 128, :],
                          in_=lsb[:])
                  koff += nkept

    nc.finalize()
    return nc


def _plan_core(idx_eff, pat, base, cover, n_tiles):
    """Sort a core's nodes by mask pattern; derive tiles and kept columns.

    Returns (gids, chunk_specs, col_ipr, col_oddm) where gids[r] is the
    global node id at permuted row r (-1 for pad rows) and col_ipr/col_oddm
    are dicts (chunk_i, tile_local, slot) -> [128] arrays.
    """
    ns_pad = n_tiles * 128
    ids = np.arange(base, base + cover)
    perm = np.argsort(PAT_RANK[pat[ids]], kind="stable")
    gids = np.full(ns_pad, -1, dtype=np.int64)
    gids[:cover] = ids[perm]

    pat_pad = np.zeros(ns_pad, dtype=np.int64)
    pat_pad[:cover] = pat[gids[:cover]]
    tile_mask = np.bitwise_or.reduce(pat_pad.reshape(n_tiles, 128), axis=1)

    nbr = np.full((ns_pad, R), N_NODES, dtype=np.int64)
    nbr[:cover] = idx_eff[gids[:cover]]

    chunk_specs = []
    col_ipr = {}
    col_oddm = {}
    t0 = 0
    chunk_i = 0
    while t0 < n_tiles:
        nt = min(CHUNK_TILES, n_tiles - t0)
        cols = []
        for s in range(1, SLOTS):
            for tl in range(nt):
                if tile_mask[t0 + tl] & (1 << (s - 1)):
                    cols.append((tl, s))
        for tl, s in cols:
            rows = nbr[(t0 + tl) * 128:(t0 + tl + 1) * 128, s - 1]
            col_ipr[(t0, tl, s)] = rows >> 1
            col_oddm[(t0, tl, s)] = (rows & 1).astype(np.uint8)
        chunk_specs.append((t0, nt, tuple(cols)))
        t0 += nt
        chunk_i += 1
    return gids, tuple(chunk_specs), col_ipr, col_oddm


def _host_inputs(node_embeddings, phrase_embeddings, score_w, score_b,
                 neighbors, neighbor_mask, node_filter_mask, n_tiles):
    """Build per-core input maps + merged chunk specs + row->node maps."""
    ns_pad = n_tiles * 128
    cover = min(NS, ns_pad)

    n32 = node_embeddings.astype(np.float32)
    nrm = np.sqrt((n32 * n32).sum(axis=1, keepdims=True))
    nhat = n32 / np.maximum(nrm, 1e-8)
    tb = nhat.astype(ml_dtypes.bfloat16)
    tpair = np.zeros((TP_ROWS, 2 * D), dtype=ml_dtypes.bfloat16)
    tpair.reshape(-1, D)[:N_NODES] = tb

    p32 = phrase_embeddings.astype(np.float32)
    prm = np.sqrt((p32 * p32).sum(axis=1, keepdims=True))
    phat = p32 / np.maximum(prm, 1e-8)
    pT = (phat.astype(ml_dtypes.bfloat16).T
          .reshape(2, 128, N_PHRASES).transpose(1, 0, 2)
          .reshape(128, 2 * N_PHRASES)).copy()

    w_p = score_w[:D].astype(np.float32)
    rest = score_w[D:].reshape(SLOTS, 2, D).astype(np.float32)
    w_n, w_c = rest[:, 0, :], rest[:, 1, :]
    wcb = np.broadcast_to(
        w_c.reshape(-1).astype(ml_dtypes.bfloat16), (128, SLOTS * D)).copy()
    wpb = np.ascontiguousarray(w_p.reshape(2, 128).T)  # [p, h]

    identb = np.eye(128, dtype=ml_dtypes.bfloat16)

    idx_eff = np.where(neighbor_mask > 0, neighbors, N_NODES).astype(np.int64)
    pat = (np.minimum(neighbor_mask, 1).astype(np.int64)
           << np.arange(R)).sum(axis=1)

    dn = nhat @ w_n.T                                     # [N, SLOTS]
    nsv = dn[:, 0].copy()
    for s in range(R):
        nsv += np.where(neighbor_mask[:, s] > 0,
                        dn[neighbors[:, s], s + 1], 0.0).astype(np.float32)
    filt = np.where(node_filter_mask > 0, 0.0, NEG_INF).astype(np.float32)
    nsf = (nsv + filt + np.float32(score_b)).astype(np.float32)

    plans = []
    for c in range(N_CORES):
        plans.append(_plan_core(idx_eff, pat, c * NS, cover, n_tiles))

    # SPMD: all cores share one program; per chunk use the union of kept
    # columns across cores (a column absent for a core gathers zero pairs)
    merged = []
    for chunk_i in range(len(plans[0][1])):
        ct0, nt, _ = plans[0][1][chunk_i]
        colset = set()
        for c in range(N_CORES):
            colset.update(plans[c][1][chunk_i][2])
        cols = tuple(sorted(colset, key=lambda ts: (ts[1], ts[0])))
        merged.append((ct0, nt, cols))
    # pyramid order: light chunks first (fast pipeline start), heavy in the
    # middle, light at the end (short drain)
    asc = sorted(merged, key=lambda e: len(e[2]))
    merged = tuple(asc[0::2] + asc[1::2][::-1])

    def wrap_idx(flat):
        blk = flat.reshape(-1, 16).T.astype(np.int16)      # [16, n/16]
        return np.tile(blk, (8, 1))

    zero_ipr = np.full(128, N_NODES >> 1, dtype=np.int64)
    zero_oddm = np.zeros(128, dtype=np.uint8)

    in_maps = []
    gid_maps = []
    for c in range(N_CORES):
        gids, _, col_ipr, col_oddm = plans[c]
        gid_maps.append(gids)

        ipr_parts = []
        oddm_parts = []
        for ct0, nt, cols in merged:
            for tl, s in cols:
                ipr_parts.append(col_ipr.get((ct0, tl, s), zero_ipr))
                oddm_parts.append(col_oddm.get((ct0, tl, s), zero_oddm))
        if ipr_parts:
            ipr_flat = np.concatenate(ipr_parts)
            oddm_cols = np.stack(oddm_parts, axis=1)
        else:
            ipr_flat = np.zeros(128, dtype=np.int64)
            oddm_cols = np.zeros((128, 1), dtype=np.uint8)

        tsf = np.zeros((ns_pad, D), dtype=tb.dtype)
        tsf[gids >= 0] = tb[gids[gids >= 0]]

        fb = np.zeros(ns_pad, dtype=np.float32)
        fb[gids >= 0] = nsf[gids[gids >= 0]]
        fb_tile = fb.reshape(n_tiles, 128).transpose(1, 0).copy()

        in_maps.append({
            "tpair": tpair,
            "tself": tsf,
            "pT": pT,
            "wcb": wcb,
            "wpb": wpb,
            "identb": identb,
            "ipr": wrap_idx(ipr_flat),
            "oddm": np.ascontiguousarray(oddm_cols),
            "nsf": np.ascontiguousarray(fb_tile),
        })
    return in_maps, merged, gid_maps


_CACHE = {}


def run_sharded(node_embeddings, phrase_embeddings, score_w, score_b,
                neighbors, neighbor_mask, node_filter_mask,
                n_tiles=None, trace=False):
    if n_tiles is None:
        n_tiles = (NS + 127) // 128  # 49
    in_maps, merged, gid_maps = _host_inputs(
        node_embeddings, phrase_embeddings, score_w, score_b,
        neighbors, neighbor_mask, node_filter_mask, n_tiles)
    key = (n_tiles, merged)
    if key not in _CACHE:
        _CACHE.clear()
        _CACHE[key] = build_kernel(n_tiles, merged)
    nc = _CACHE[key]
    res = run_bass_kernel_spmd(nc, in_maps, list(range(N_CORES)), trace=trace)

    full_t = np.zeros((N_NODES, N_PHRASES), dtype=np.float32)
    for c in range(N_CORES):
        gids = gid_maps[c]
        valid = gids >= 0
        full_t[gids[valid]] = res.results[c]["out"][valid]
    out = np.ascontiguousarray(full_t.T)
    return out, res


def kernel(node_embeddings, phrase_embeddings, score_w, score_b,
           neighbors, neighbor_mask, node_filter_mask):
    out, _ = run_sharded(
        np.asarray(node_embeddings, dtype=np.float32),
        np.asarray(phrase_embeddings, dtype=np.float32),
        np.asarray(score_w, dtype=np.float32),
        np.asarray(score_b, dtype=np.float32),
        np.asarray(neighbors),
        np.asarray(neighbor_mask),
        np.asarray(node_filter_mask))
    return out
